# revision 5
# baseline (speedup 1.0000x reference)
"""MixHop GNN kernel for one TRN2 chip (8 NeuronCores), Bass/Tile.

Math (matches the reference exactly):
    row/col = edge_index with self loops appended
    deg[t]  = #edges with col==t          (host: integer bincount)
    dinv    = 1/sqrt(deg)
    prop(h) = D^-1/2 (A+I) D^-1/2 h
            -> z = dinv*h; y[t] = sum_{edges s->t} z[s] (self loop is an
               explicit edge); h_out = dinv*y
    h1 = prop(x); h2 = prop(h1)
    out = relu(concat(x@W0+b0, h1@W1+b1, h2@W2+b2)) @ Wout + bout

Layout: FEATURE-MAJOR. Core c owns targets [c*N/8, (c+1)*N/8), permuted
by ascending degree so each 128-target window has a homogeneous slot
count D. The z table lives in SBUF as [128 feats, node] f32, in lo/hi
halves of 25000 columns (ap_gather num_elems limit). Per (window, phase)
one ap_gather pulls the D-padded neighbor slots (pads hit a zeroed
column) for all 128 features at once, and one strided tensor_reduce
produces the 128 per-target sums. z0 = dinv*x is computed on host and
shipped as the two phase tables; z1 is exchanged with one f32 AllGather
(kept in permuted order -- prop-2 gather indices compose the
permutation, precomputed on host). The head is transpose-free off the
feature-major h tiles.
"""
import numpy as np
import ml_dtypes

N = 50000
F = 128
NCORE = 8
NPC = N // NCORE          # 6250 targets per core
WIN = 128
NWIN = (NPC + WIN - 1) // WIN      # 49
NPAD = NWIN * WIN                  # 6272
HALF = 25000                       # nodes per table phase
TBLC = HALF + 4                    # + zeroed pad columns
ZCOL = HALF                        # pad slots gather this (zero) column
PER_HOP = 64
OUT = 64
MAXIDX = 4096                      # max idxs per ap_gather call


def _wrap16(flat):
    return np.tile(np.asarray(flat, dtype=np.int16).reshape(-1, 16).T, (8, 1))


def _preprocess(edge_index):
    row = np.asarray(edge_index[0], dtype=np.int64)
    col = np.asarray(edge_index[1], dtype=np.int64)
    rows = np.concatenate([row, np.arange(N, dtype=np.int64)])
    cols = np.concatenate([col, np.arange(N, dtype=np.int64)])
    deg = np.bincount(cols, minlength=N).astype(np.float64)
    dinv = (1.0 / np.sqrt(deg)).astype(np.float32)

    orders, ranks, nbrs = [], [], []
    dmax = np.zeros((NWIN, 2), dtype=np.int64)
    for c in range(NCORE):
        lo, hi = c * NPC, (c + 1) * NPC
        sel = (cols >= lo) & (cols < hi)
        r, t = rows[sel], cols[sel] - lo
        dtot = np.bincount(t, minlength=NPC)
        order = np.argsort(dtot, kind="stable")      # ascending degree
        rank = np.empty(NPC, dtype=np.int64)
        rank[order] = np.arange(NPC)
        tp = rank[t]
        # bucket edges per (phase, permuted target); keep raw src ids
        o_ph = (r >= HALF).astype(np.int64)
        nbr = [[[] for _ in range(NPC)], [[] for _ in range(NPC)]]
        for src, tgt, h in zip(r, tp, o_ph):
            nbr[h][tgt].append(src)
        dm = np.zeros((NWIN, 2), dtype=np.int64)
        for hh in range(2):
            cnt = np.array([len(x) for x in nbr[hh]])
            for w in range(NWIN):
                seg = cnt[w * WIN:min((w + 1) * WIN, NPC)]
                dm[w, hh] = seg.max() if len(seg) else 0
        dmax = np.maximum(dmax, dm)
        orders.append(order)
        ranks.append(rank)
        nbrs.append(nbr)

    Ds = tuple((int(dmax[w, 0]), int(dmax[w, 1])) for w in range(NWIN))
    ncols = sum(WIN * (d0 + d1) // 16 for d0, d1 in Ds)

    # prop-2 gather position of raw node s (permuted z1 table layout)
    pos2 = np.empty(N, dtype=np.int64)
    for c in range(NCORE):
        pos2[c * NPC:(c + 1) * NPC] = c * NPC + ranks[c]
    # table-local column for each raw node, per prop
    # prop1: node s -> col s - h*HALF ; prop2: node s -> pos2[s] - h*HALF

    percore = []
    for c in range(NCORE):
        nbr = nbrs[c]
        idx16 = np.empty((128, 2 * ncols), dtype=np.int16)
        for prop in range(2):
            off = prop * ncols
            for w in range(NWIN):
                for hh in range(2):
                    D = Ds[w][hh]
                    if D == 0:
                        continue
                    blk = np.full((WIN, D), ZCOL, dtype=np.int64)
                    for j in range(WIN):
                        p = w * WIN + j
                        if p < NPC:
                            lst = nbr[hh][p]
                            if lst:
                                a = np.asarray(lst, dtype=np.int64)
                                if prop == 1:
                                    a = pos2[a]
                                blk[j, :len(a)] = a - hh * HALF
                    flat = blk.reshape(-1)
                    wcols = len(flat) // 16
                    idx16[:, off:off + wcols] = _wrap16(flat)
                    off += wcols
        dvt = np.zeros(NPAD, dtype=np.float16)
        dvt[:NPC] = dinv[c * NPC + orders[c]]
        percore.append({
            "idx16": np.ascontiguousarray(idx16),
            "order": orders[c],
            "dinv_t": np.ascontiguousarray(
                np.broadcast_to(dvt, (128, NPAD)).copy()),
        })
    return Ds, percore, dinv


def _build(Ds):
    import concourse.bass as bass  # noqa: F401
    import concourse.bacc as bacc
    import concourse.tile as tile
    import concourse.mybir as mybir

    dt = mybir.dt
    f32 = dt.float32
    f16 = dt.float16
    bf16 = dt.bfloat16
    AF = mybir.ActivationFunctionType
    ALU = mybir.AluOpType

    icols = [WIN * (d0 + d1) // 16 for d0, d1 in Ds]
    IOFF = np.concatenate([[0], np.cumsum(icols)]).astype(int)
    NC1 = int(IOFF[-1])                     # idx cols per prop

    nc = bacc.Bacc("TRN2", target_bir_lowering=False, debug=False,
                   num_devices=NCORE)

    z0lo_in = nc.dram_tensor("z0lo", [128, TBLC], f32, kind="ExternalInput")
    z0hi_in = nc.dram_tensor("z0hi", [128, TBLC], f32, kind="ExternalInput")
    idx_in = nc.dram_tensor("idx16", [128, 2 * NC1], dt.int16,
                            kind="ExternalInput")
    xt_in = nc.dram_tensor("xt", [128, NPAD], bf16, kind="ExternalInput")
    dv_in = nc.dram_tensor("dinv_t", [128, NPAD], f16, kind="ExternalInput")
    w_in = [nc.dram_tensor(f"w{k}", [F, PER_HOP], bf16,
                           kind="ExternalInput") for k in range(3)]
    wo_in = nc.dram_tensor("wout", [3 * PER_HOP, OUT], bf16,
                           kind="ExternalInput")
    b_in = [nc.dram_tensor(f"b{k}", [PER_HOP, 1], f32,
                           kind="ExternalInput") for k in range(3)]
    bo_in = nc.dram_tensor("bout", [OUT, 1], f32, kind="ExternalInput")
    out_t = nc.dram_tensor("out_t", [OUT, NPC], f32, kind="ExternalOutput")

    z1b = nc.dram_tensor("z1b", [128, NPC], f32)
    z1f = nc.dram_tensor("z1f", [128 * NCORE, NPC], f32, addr_space="Shared")

    def ws(w):
        return slice(w * WIN, (w + 1) * WIN)

    with tile.TileContext(nc) as tc:
        with (
            tc.tile_pool(name="persist", bufs=1) as pp,
            tc.tile_pool(name="idxp", bufs=3) as ip,
            tc.tile_pool(name="gout", bufs=2) as gp,
            tc.tile_pool(name="hx", bufs=2) as hp,
            tc.tile_pool(name="zst", bufs=2) as zp,
            tc.tile_pool(name="psum", bufs=2, space="PSUM") as ps,
        ):
            # ---- persistent ----
            table = pp.tile([128, TBLC], f32)
            nc.vector.memset(table[:, HALF:], 0.0)
            dinv_sb = pp.tile([128, NPAD], f16)
            nc.sync.dma_start(out=dinv_sb[:], in_=dv_in[:])
            y_acc = pp.tile([128, NPAD], f32)
            h1t = pp.tile([128, NPAD], bf16)
            w_sb = []
            for k in range(3):
                t = pp.tile([F, PER_HOP], bf16, tag=f"w{k}")
                nc.sync.dma_start(out=t[:], in_=w_in[k][:])
                w_sb.append(t)
            wo_sb = []
            for k in range(3):
                t = pp.tile([PER_HOP, OUT], bf16, tag=f"wo{k}")
                nc.sync.dma_start(
                    out=t[:], in_=wo_in.ap()[k * PER_HOP:(k + 1) * PER_HOP, :])
                wo_sb.append(t)
            b_sb = []
            for k in range(3):
                t = pp.tile([PER_HOP, 1], f32, tag=f"b{k}")
                nc.sync.dma_start(out=t[:], in_=b_in[k][:])
                b_sb.append(t)
            bo_sb = pp.tile([OUT, 1], f32)
            nc.sync.dma_start(out=bo_sb[:], in_=bo_in[:])
            rx = pp.tile([PER_HOP, NPAD], bf16)
            r1 = pp.tile([PER_HOP, NPAD], bf16)

            def load_table_z0(src):
                nc.sync.dma_start(out=table[:, 0:HALF],
                                  in_=src.ap()[:, 0:HALF])

            def load_table_z1(h):
                nc.sync.dma_start(
                    out=table[:, 0:HALF].rearrange("p (c n) -> p c n", n=NPC),
                    in_=z1f.ap()[4 * h * 128:4 * (h + 1) * 128, :].rearrange(
                        "(c p) n -> p c n", p=128),
                )

            def prop_phase(prop, h, first):
                for w in range(NWIN):
                    D = Ds[w][h]
                    if D == 0:
                        continue
                    nidx = WIN * D
                    coff = (prop * NC1 + int(IOFF[w])
                            + (0 if h == 0 else WIN * Ds[w][0] // 16))
                    wcols = nidx // 16
                    idxb = ip.tile([128, wcols], dt.int16, tag="idx")
                    nc.sync.dma_start(out=idxb[:],
                                      in_=idx_in.ap()[:, coff:coff + wcols])
                    g = gp.tile([128, nidx], f32, tag="g")
                    chunk = (MAXIDX // (16 * D)) * 16 * D
                    assert chunk > 0, f"D={D} too large"
                    done = 0
                    while done < nidx:
                        n = min(nidx - done, chunk)
                        nc.gpsimd.ap_gather(
                            out_ap=g[:, done:done + n].rearrange(
                                "p (n d) -> p n d", d=1),
                            in_ap=table[:].rearrange("p (n d) -> p n d", d=1),
                            idxs_ap=idxb[:, done // 16:(done + n) // 16],
                            channels=128, num_elems=TBLC, d=1, num_idxs=n)
                        done += n
                    if first:
                        nc.vector.tensor_reduce(
                            out=y_acc[:, ws(w)],
                            in_=g[:].rearrange("p (t d) -> p t d", d=D),
                            axis=mybir.AxisListType.X, op=ALU.add)
                    else:
                        red = gp.tile([128, WIN], f32, tag="red")
                        nc.vector.tensor_reduce(
                            out=red[:],
                            in_=g[:].rearrange("p (t d) -> p t d", d=D),
                            axis=mybir.AxisListType.X, op=ALU.add)
                        nc.vector.tensor_tensor(
                            out=y_acc[:, ws(w)], in0=y_acc[:, ws(w)],
                            in1=red[:], op=ALU.add)

            # ---- prop 1 (z0 tables from host) ----
            load_table_z0(z0lo_in)
            prop_phase(0, 0, first=True)
            load_table_z0(z0hi_in)
            prop_phase(0, 1, first=False)

            # h1 = dinv * y ; z1 = dinv * h1 -> bounce (permuted) -> AllGather
            nc.vector.tensor_tensor(out=h1t[:], in0=y_acc[:], in1=dinv_sb[:],
                                    op=ALU.mult)
            CH = 625
            for i in range(NPC // CH):
                zs = zp.tile([128, CH], f32, tag="zs")
                nc.vector.tensor_tensor(
                    out=zs[:], in0=h1t[:, i * CH:(i + 1) * CH],
                    in1=dinv_sb[:, i * CH:(i + 1) * CH], op=ALU.mult)
                nc.sync.dma_start(out=z1b.ap()[:, i * CH:(i + 1) * CH],
                                  in_=zs[:])
            nc.gpsimd.collective_compute(
                "AllGather", ALU.bypass,
                replica_groups=[list(range(NCORE))],
                ins=[z1b[:]], outs=[z1f[:]])

            # overlap with AllGather: x and h1 head hops (PE + scalar)
            for w in range(NWIN):
                xw = hp.tile([128, WIN], bf16, tag="xw")
                nc.sync.dma_start(out=xw[:], in_=xt_in.ap()[:, ws(w)])
                cps = ps.tile([PER_HOP, WIN], f32, tag="cps")
                nc.tensor.matmul(out=cps[:], lhsT=w_sb[0][:], rhs=xw[:],
                                 start=True, stop=True)
                nc.scalar.activation(out=rx[:, ws(w)], in_=cps[:],
                                     func=AF.Relu, bias=b_sb[0][:])
                cps1 = ps.tile([PER_HOP, WIN], f32, tag="cps1")
                nc.tensor.matmul(out=cps1[:], lhsT=w_sb[1][:],
                                 rhs=h1t[:, ws(w)], start=True, stop=True)
                nc.scalar.activation(out=r1[:, ws(w)], in_=cps1[:],
                                     func=AF.Relu, bias=b_sb[1][:])

            # ---- prop 2 (z1 tables, permuted layout) ----
            load_table_z1(0)
            prop_phase(1, 0, first=True)
            load_table_z1(1)
            prop_phase(1, 1, first=False)

            # ---- finish head per window ----
            for w in range(NWIN):
                h2w = hp.tile([128, WIN], bf16, tag="h2w")
                nc.vector.tensor_tensor(out=h2w[:], in0=y_acc[:, ws(w)],
                                        in1=dinv_sb[:, ws(w)], op=ALU.mult)
                cps2 = ps.tile([PER_HOP, WIN], f32, tag="c2")
                nc.tensor.matmul(out=cps2[:], lhsT=w_sb[2][:], rhs=h2w[:],
                                 start=True, stop=True)
                r2 = hp.tile([PER_HOP, WIN], bf16, tag="r2")
                nc.scalar.activation(out=r2[:], in_=cps2[:],
                                     func=AF.Relu, bias=b_sb[2][:])
                ops = ps.tile([OUT, WIN], f32, tag="ops")
                nc.tensor.matmul(out=ops[:], lhsT=wo_sb[0][:],
                                 rhs=rx[:, ws(w)], start=True, stop=False)
                nc.tensor.matmul(out=ops[:], lhsT=wo_sb[1][:],
                                 rhs=r1[:, ws(w)], start=False, stop=False)
                nc.tensor.matmul(out=ops[:], lhsT=wo_sb[2][:],
                                 rhs=r2[:], start=False, stop=True)
                ow = hp.tile([OUT, WIN], f32, tag="ow")
                nc.scalar.activation(out=ow[:], in_=ops[:],
                                     func=AF.Identity, bias=bo_sb[:])
                lim = min(NPC, (w + 1) * WIN) - w * WIN
                nc.sync.dma_start(out=out_t.ap()[:, w * WIN:w * WIN + lim],
                                  in_=ow[:, 0:lim])

    nc.compile()
    return nc


_CACHE = {}


def _get_nc(Ds):
    if Ds not in _CACHE:
        _CACHE[Ds] = _build(Ds)
    return _CACHE[Ds]


def make_in_maps(x, percore, dinv, W0, b0, W1, b1, W2, b2, Wout, bout):
    x = np.asarray(x, dtype=np.float32)
    z0 = x * dinv[:, None]                      # [N, F] f32, host-exact
    z0lo = np.zeros((128, TBLC), dtype=np.float32)
    z0lo[:, :HALF] = z0[:HALF].T
    z0hi = np.zeros((128, TBLC), dtype=np.float32)
    z0hi[:, :N - HALF] = z0[HALF:].T
    common = {
        "z0lo": np.ascontiguousarray(z0lo),
        "z0hi": np.ascontiguousarray(z0hi),
        "w0": np.asarray(W0).astype(ml_dtypes.bfloat16),
        "w1": np.asarray(W1).astype(ml_dtypes.bfloat16),
        "w2": np.asarray(W2).astype(ml_dtypes.bfloat16),
        "wout": np.asarray(Wout).astype(ml_dtypes.bfloat16),
        "b0": np.asarray(b0, dtype=np.float32).reshape(PER_HOP, 1),
        "b1": np.asarray(b1, dtype=np.float32).reshape(PER_HOP, 1),
        "b2": np.asarray(b2, dtype=np.float32).reshape(PER_HOP, 1),
        "bout": np.asarray(bout, dtype=np.float32).reshape(OUT, 1),
    }
    in_maps = []
    for c in range(NCORE):
        pc = percore[c]
        xp = np.zeros((NPAD, F), dtype=np.float32)
        xp[:NPC] = x[c * NPC:(c + 1) * NPC][pc["order"]]
        m = dict(common)
        m["idx16"] = pc["idx16"]
        m["dinv_t"] = pc["dinv_t"]
        m["xt"] = np.ascontiguousarray(xp.T.astype(ml_dtypes.bfloat16))
        in_maps.append(m)
    return in_maps


def run(inputs, trace=False):
    from concourse.bass_utils import run_bass_kernel_spmd

    Ds, percore, dinv = _preprocess(np.asarray(inputs["edge_index"]))
    nc = _get_nc(Ds)
    in_maps = make_in_maps(
        inputs["x"], percore, dinv, inputs["W0"], inputs["b0"],
        inputs["W1"], inputs["b1"], inputs["W2"], inputs["b2"],
        inputs["Wout"], inputs["bout"])
    res = run_bass_kernel_spmd(nc, in_maps, core_ids=list(range(NCORE)),
                               trace=trace)
    out = np.empty((N, OUT), dtype=np.float32)
    for c in range(NCORE):
        o = np.asarray(res.results[c]["out_t"]).T    # [NPC, OUT] permuted
        out[c * NPC + percore[c]["order"]] = o
    return out, res


def kernel(x, edge_index, W0, b0, W1, b1, W2, b2, Wout, bout):
    out, _ = run({"x": x, "edge_index": edge_index, "W0": W0, "b0": b0,
                  "W1": W1, "b1": b1, "W2": W2, "b2": b2,
                  "Wout": Wout, "bout": bout})
    return out
